# revision 3
# baseline (speedup 1.0000x reference)
"""Bass kernel v4: per-chunk psum, balanced engines, tight prologue/tail.

Per-core (B sharded 8 ways): enc [64, 4, 704] -> 6 heads [64, 4, 256];
s[i,j,b,d] = q_i*k_j; softmax_j; out_i = sum_j a_ij v_j.

Structure per chunk g (32 chunks: t, c2 = divmod(g, 8); p = t//2, h = t%2):
  [prefill (even, DVE 4x) + Pool accum-mult qbc] or [SP qbc + DVE S-TT]
  -> exp (Act) -> EV (Pool or DVE) -> 8 reduce MMs (PE) into ps4[g%4]
  -> evac [98, 1024] (Act late or DVE) -> znscr write (SP, padded 258 runs)
Epilogue: 16 strided reloads (8 early / 8 late, SP+Pool) -> recip+mul -> out.

e/ev layout [(b2)(j), (i8:8)(d)]; reduce Z rows {64rp,+1}, N {64rp+32,+33},
cols cb*512; znscr[c] = 392 runs of 258 from [98, 1024].
z_all/n_all [128, 1024]: P = rp*64 + b2*32 + c_half*16 + (c%16), wait -
P = rp*64 + b2*32 + c%... see reload_src; f = cb*512 + i2*256 + d,
i8 = 2*(2*cb+rp) + i2.
"""

import sys

sys.path.insert(0, "/opt/trn_rl_repo")
import numpy as np
import concourse.bass as bass
from concourse import mybir
from concourse.alu_op_type import AluOpType

F32 = mybir.dt.float32
BF16 = mybir.dt.bfloat16
AF = mybir.ActivationFunctionType

KT = 6
NCH = 32
CHF = 2048
NGRP = 12
ZNROW = 258
ZNC = 392 * ZNROW          # znscr elems per chunk

EV_DVE = {4, 9, 14, 20, 25, 30}                  # 6 on DVE, 26 Pool
EVAC_ACT = {1, 5, 9, 13, 17, 21, 25, 29, 30}      # on Act (late), rest DVE

# proj head order: q1 first (qscr h0), then k1/v1 for first chunks
ALL_HEADS = ["wq1", "wk1", "wv1", "wq2", "wk2", "wv2"]


def ev_dve_prefix(g):
    return sum(1 for x in EV_DVE if x <= g)


def ev_pool_prefix(g):
    return sum(1 for x in range(NCH) if x not in EV_DVE and x <= g)


def evac_act_prefix(c):
    return sum(1 for x in EVAC_ACT if x <= c)


def evac_dve_prefix(c):
    return sum(1 for x in range(NCH) if x not in EVAC_ACT and x <= c)


def off(mt, h):
    return (mt * 2 + h) * 256


def build_nc():
    nc = bass.Bass()

    encb = nc.declare_dram_parameter("encb", [768, 256], BF16, isOutput=False)
    enca = nc.declare_dram_parameter("enca", [768, 256], BF16, isOutput=False)
    ws = {}
    for wname in ALL_HEADS:
        ws[wname] = nc.declare_dram_parameter(wname, [768, 256], BF16, isOutput=False)
    redsel = nc.declare_dram_parameter("redsel", [128, 2], BF16, isOutput=False)
    out_ext = nc.declare_dram_parameter("out", [128, 1024], F32, isOutput=True)

    qscr = nc.dram_tensor("qscr", [2, 2, 2, 64, 256], BF16)   # [h, p, b2, i, d]
    znscr = nc.dram_tensor("znscr", [NCH, ZNC], BF16)

    from contextlib import ExitStack
    with ExitStack() as stack:
        en = stack.enter_context
        encb_sb = en(nc.sbuf_tensor([128, KT * 256], BF16))
        enca_sb = en(nc.sbuf_tensor([128, KT * 256], BF16))
        w_sb = en(nc.sbuf_tensor([128, 6 * KT * 256], BF16))
        redsel_sb = en(nc.sbuf_tensor([128, 2], BF16))
        k_sb = en(nc.sbuf_tensor([128, 4 * 256], BF16))
        v_sb = en(nc.sbuf_tensor([128, 4 * 256], BF16))
        q_sb = en(nc.sbuf_tensor([128, 4 * 256], BF16))
        qbc_sb = en(nc.sbuf_tensor([128, 2 * CHF], BF16))
        qbc2_sb = en(nc.sbuf_tensor([128, 2 * CHF], BF16))
        s_sb = en(nc.sbuf_tensor([128, 4 * CHF], BF16))
        e_sb = en(nc.sbuf_tensor([128, 4 * CHF], BF16))
        ev_sb = en(nc.sbuf_tensor([128, 4 * CHF], BF16))
        zn_sb = en(nc.sbuf_tensor([98, 4 * 1024], BF16))
        z_all = en(nc.sbuf_tensor([128, 1024], BF16))
        n_all = en(nc.sbuf_tensor([128, 1024], BF16))
        rcp_sb = en(nc.sbuf_tensor([128, 1024], F32))
        outp_sb = en(nc.sbuf_tensor([128, 1024], F32))
        ps4 = [en(nc.psum_tensor(f"ps4_{i}", [128, 1024], F32))
               for i in range(4)]

        dma_ea = en(nc.semaphore("dma_ea"))      # enca load
        dma_eb = en(nc.semaphore("dma_eb"))      # encb load
        dma_rs = en(nc.semaphore("dma_rs"))      # redsel load
        dma_w = {}
        for _wn in ALL_HEADS:
            dma_w[_wn] = en(nc.semaphore(f"dma_w_{_wn}"))
        dmaq0 = en(nc.semaphore("dmaq0"))        # qscr writes h=0
        dmaq1 = en(nc.semaphore("dmaq1"))        # qscr writes h=1
        qdma_o0 = en(nc.semaphore("qdma_o0"))
        qdma_o1 = en(nc.semaphore("qdma_o1"))
        qdma_e0 = en(nc.semaphore("qdma_e0"))
        qdma_e1 = en(nc.semaphore("qdma_e1"))
        pfill = en(nc.semaphore("pfill"))
        peproj = en(nc.semaphore("peproj"))
        aevac = en(nc.semaphore("aevac"))
        vstt = en(nc.semaphore("vstt"))
        scexp = en(nc.semaphore("scexp"))
        vev_p = en(nc.semaphore("vev_p"))
        vev_d = en(nc.semaphore("vev_d"))
        pered = en(nc.semaphore("pered"))
        sevac_a = en(nc.semaphore("sevac_a"))
        sevac_d = en(nc.semaphore("sevac_d"))
        dmazn_l = [en(nc.semaphore(f"dmazn{_i}")) for _i in range(4)]
        dmaep = en(nc.semaphore("dmaep"))        # SP z reloads
        dmaep_n = en(nc.semaphore("dmaep_n"))    # SP n reloads
        dmaep_p = en(nc.semaphore("dmaep_p"))    # Pool z reloads
        dmaep_pn = en(nc.semaphore("dmaep_pn"))  # Pool n reloads
        vep = en(nc.semaphore("vep"))
        vmz = en(nc.semaphore("vmz"))
        dmaout = en(nc.semaphore("dmaout"))
        block = en(nc.Block())

        qdma_o = [qdma_o0, qdma_o1]
        qdma_e = [qdma_e0, qdma_e1]
        dmazn = dmazn_l
        dmaqh = [dmaq0, dmaq1]
        # proj psum buffers alias ps4[2]/ps4[3] (chunks 2,3 gated on aevac=12)
        ps_projs = [ps4[2][:, 0:256], ps4[2][:, 512:768],
                    ps4[3][:, 0:256], ps4[3][:, 512:768]]

        def chunk_params(g):
            t, c2 = divmod(g, 8)
            return t // 2, t % 2, c2

        def k_grp(g):
            p, h, _ = chunk_params(g)
            return h * 6 + 2 + p + 1          # k1 groups 2,3; k2 groups 8,9

        def v_grp(g):
            p, h, _ = chunk_params(g)
            return h * 6 + 4 + p + 1          # v1 groups 4,5; v2 groups 10,11

        def qbc_src(g):
            p, h, c2 = chunk_params(g)
            base = qscr[h, p]
            return bass.AP(
                tensor=base.tensor,
                offset=base.offset + c2 * 8 * 256,
                ap=[[64 * 256, 2], [0, 64], [1, 2048]],
            )

        def znw_dst(c):
            base = znscr[c]
            return bass.AP(tensor=base.tensor, offset=base.offset,
                           ap=[[ZNROW, 392], [1, 256]])

        # reload for (rp, tsel, b2, half): partitions P = rp*64+b2*32+c,
        # c in [half*16, half*16+16); src flat per c:
        # ZNC*c + ((64rp+32tsel+b2)*4 + cb*2 + i2)*258 + d
        def reload_src(rp, tsel, b2, c0, n):
            s = 64 * rp + 32 * tsel + b2
            base = znscr[c0]
            return bass.AP(
                tensor=base.tensor,
                offset=base.offset + s * 4 * ZNROW,
                ap=[[ZNC, n], [ZNROW, 4], [1, 256]],
            )

        # batch boundaries for incremental reloads
        RL = [(0, 16), (16, 28), (28, 32)]

        def emit_reload_piece(eng, sem, bi, tsel, rp, b2sel):
            c0, c1 = RL[bi]
            dst_all = z_all if tsel == 0 else n_all
            p0 = rp * 64 + b2sel * 32 + c0
            eng.dma_start(out=dst_all[p0:p0 + (c1 - c0), :],
                          in_=reload_src(rp, tsel, b2sel, c0, c1 - c0)
                          ).then_inc(sem, 16)

        def emit_reload_batch(eng, sems, bi, b2sel):
            for tsel in range(2):
                for rp in range(2):
                    emit_reload_piece(eng, sems[tsel], bi, tsel, rp, b2sel)

        # pieces for batches 0 and 1, interleaved into SP slots c=24..31
        RL_PIECES = [(bi, tsel, rp, b2sel)
                     for bi in range(2)
                     for tsel in range(2)
                     for rp in range(2)
                     for b2sel in range(2)]

        def wait_znw_upto(eng, cmax):
            for s4 in range(4):
                cnt = sum(1 for x in range(cmax + 1) if x % 4 == s4)
                eng.wait_ge(dmazn[s4], 16 * cnt)

        # ---------------- sync (SP) ----------------
        @block.sync
        def _(sync):
            sync.dma_start(out=enca_sb[:, :].rearrange("p (kt d) -> p kt d", kt=KT),
                           in_=bass.AP(tensor=enca[0, 0].tensor, offset=0,
                                       ap=[[256, 128], [128 * 256, KT], [1, 256]])
                           ).then_inc(dma_ea, 16)
            sync.dma_start(out=encb_sb[:, :].rearrange("p (kt d) -> p kt d", kt=KT),
                           in_=bass.AP(tensor=encb[0, 0].tensor, offset=0,
                                       ap=[[256, 128], [128 * 256, KT], [1, 256]])
                           ).then_inc(dma_eb, 16)
            sync.dma_start(out=redsel_sb[:, :], in_=redsel[:, :]).then_inc(dma_rs, 16)

            # qscr writes (h, mt): need q-proj evac group h*6+mt
            def emit_qscr(h, mt):
                sync.wait_ge(aevac, h * 6 + mt + 1)
                base = qscr[h, 0]
                dst = bass.AP(tensor=base.tensor,
                              offset=base.offset + mt * 32 * 256,
                              ap=[[256, 32], [64 * 256, 4], [1, 256]])
                sync.dma_start(out=dst, in_=q_sb[:, off(mt, h):off(mt, h) + 256]
                               ).then_inc(dmaqh[h], 16)

            emit_qscr(0, 0)
            emit_qscr(0, 1)

            def emit_qbc_odd(g):
                gbuf = (g // 2) % 2
                if g >= 5:
                    sync.wait_ge(vstt, g - 3)
                h = chunk_params(g)[1]
                sync.wait_ge(dmaqh[h], 32)
                sync.dma_start(out=qbc_sb[:, gbuf * CHF:(gbuf + 1) * CHF],
                               in_=qbc_src(g)).then_inc(qdma_o[gbuf], 16)

            emit_qbc_odd(1)
            emit_qbc_odd(3)
            emit_qbc_odd(5)
            emit_qscr(1, 0)
            emit_qscr(1, 1)
            for c in range(NCH):
                if c + 7 < NCH and (c + 7) % 2 == 1:
                    emit_qbc_odd(c + 7)
                if c >= 2:
                    cz = c - 2
                    if cz in EVAC_ACT:
                        sync.wait_ge(sevac_a, evac_act_prefix(cz))
                    else:
                        sync.wait_ge(sevac_d, evac_dve_prefix(cz))
                    sync.dma_start(out=znw_dst(cz),
                                   in_=zn_sb[:, (cz % 4) * 1024:(cz % 4) * 1024 + 1024]
                                   ).then_inc(dmazn[cz % 4], 16)
                if c == 25:
                    wait_znw_upto(sync, 15)
                    emit_reload_batch(sync, (dmaep, dmaep_n), 0, 0)
                    emit_reload_batch(sync, (dmaep, dmaep_n), 0, 1)
            wait_znw_upto(sync, 27)
            emit_reload_batch(sync, (dmaep, dmaep_n), 1, 0)
            for cz in (30, 31):
                if cz in EVAC_ACT:
                    sync.wait_ge(sevac_a, evac_act_prefix(cz))
                else:
                    sync.wait_ge(sevac_d, evac_dve_prefix(cz))
                sync.dma_start(out=znw_dst(cz),
                               in_=zn_sb[:, (cz % 4) * 1024:(cz % 4) * 1024 + 1024]
                               ).then_inc(dmazn[cz % 4], 16)
            wait_znw_upto(sync, 31)
            emit_reload_batch(sync, (dmaep, dmaep_n), 2, 0)

            sync.wait_ge(vep, 3)
            sync.dma_start(out=out_ext[:, 0:512], in_=outp_sb[:, 0:512]
                           ).then_inc(dmaout, 16)
            sync.wait_ge(vep, 6)
            sync.dma_start(out=out_ext[:, 512:1024], in_=outp_sb[:, 512:1024]
                           ).then_inc(dmaout, 16)

        def wait_ev_a(eng, g):
            eng.wait_ge(vev_d, ev_dve_prefix(g))
            eng.wait_ge(vev_p, ev_pool_prefix(g))

        # ---------------- scalar (Act): w loads, proj evacs, exp, late evacs --
        @block.scalar
        def _(scalar):
            for wname in ["wq1"]:
                scalar.dma_start(
                    out=w_sb[:, ALL_HEADS.index(wname) * KT * 256:][:, :KT * 256]
                        .rearrange("p (kt d) -> p kt d", kt=KT),
                    in_=bass.AP(tensor=ws[wname][0, 0].tensor, offset=0,
                                ap=[[256, 128], [128 * 256, KT], [1, 256]])
                    ).then_inc(dma_w[wname], 16)
            # warm the activation table before the evac/exp stream
            _cz = nc.const_aps.scalar_like(0.0, rcp_sb[0:1, 0:1])
            nc.scalar.activation(rcp_sb[0:1, 0:1], _cz, AF.Exp)

            def emit_proj_evac(grp):
                gi, mt = grp // 2, grp % 2
                wname = ALL_HEADS[gi]
                scalar.wait_ge(peproj, grp + 1)
                buf = ps_projs[grp % 4]
                if wname in ("wk1", "wk2"):
                    dst = k_sb[:, off(mt, 0 if wname == "wk1" else 1):][:, :256]
                elif wname in ("wv1", "wv2"):
                    dst = v_sb[:, off(mt, 0 if wname == "wv1" else 1):][:, :256]
                else:
                    dst = q_sb[:, off(mt, 0 if wname == "wq1" else 1):][:, :256]
                nc.scalar.copy(dst, buf).then_inc(aevac, 1)

            for grp in range(6):
                emit_proj_evac(grp)

            for g in range(NCH):
                if g < 6:
                    emit_proj_evac(6 + g)
                scalar.wait_ge(vstt, g + 1)
                if g >= 4:
                    scalar.wait_ge(pered, g - 3)
                nc.scalar.activation(
                    e_sb[:, (g % 4) * CHF:(g % 4 + 1) * CHF],
                    s_sb[:, (g % 4) * CHF:(g % 4 + 1) * CHF],
                    AF.Exp,
                ).then_inc(scexp, 1)
                # late-emitted Act evacs: chunk c = g - 4
                if g >= 4 and (g - 4) in EVAC_ACT:
                    c = g - 4
                    scalar.wait_ge(pered, c + 1)
                    if c >= 4:
                        scalar.wait_ge(dmazn[c % 4], 16 * (c // 4))
                    nc.scalar.copy(
                        zn_sb[:, (c % 4) * 1024:(c % 4) * 1024 + 1024],
                        ps4[c % 4][0:98, :],
                    ).then_inc(sevac_a, 1)
            for c in range(NCH - 4, NCH):
                if c in EVAC_ACT:
                    scalar.wait_ge(pered, c + 1)
                    scalar.wait_ge(dmazn[c % 4], 16 * (c // 4))
                    nc.scalar.copy(
                        zn_sb[:, (c % 4) * 1024:(c % 4) * 1024 + 1024],
                        ps4[c % 4][0:98, :],
                    ).then_inc(sevac_a, 1)

        # ---------------- gpsimd (Pool) ----------------
        @block.gpsimd
        def _(gpsimd):
            for wname in ["wk1", "wv1", "wq2", "wk2", "wv2"]:
                gpsimd.dma_start(
                    out=w_sb[:, ALL_HEADS.index(wname) * KT * 256:][:, :KT * 256]
                        .rearrange("p (kt d) -> p kt d", kt=KT),
                    in_=bass.AP(tensor=ws[wname][0, 0].tensor, offset=0,
                                ap=[[256, 128], [128 * 256, KT], [1, 256]])
                    ).then_inc(dma_w[wname], 16)

            def emit_qbc_even(g):
                h = chunk_params(g)[1]
                gpsimd.wait_ge(dmaqh[h], 32)
                if g >= 4:
                    gpsimd.wait_ge(vstt, g - 3)
                gbuf = (g // 2) % 2
                gpsimd.dma_start(out=qbc2_sb[:, gbuf * CHF:(gbuf + 1) * CHF],
                                 in_=qbc_src(g)
                                 ).then_inc(qdma_e[gbuf], 16)

            emit_qbc_even(0)
            emit_qbc_even(2)
            for g in range(NCH):
                if g not in EV_DVE:
                    p, h, c2 = chunk_params(g)
                    gpsimd.wait_ge(aevac, v_grp(g))
                    gpsimd.wait_ge(scexp, g + 1)
                    if g >= 4:
                        gpsimd.wait_ge(pered, g - 3)
                    nc.gpsimd.tensor_mul(
                        ev_sb[:, (g % 4) * CHF:(g % 4 + 1) * CHF]
                            .rearrange("p (i d) -> p i d", i=8),
                        e_sb[:, (g % 4) * CHF:(g % 4 + 1) * CHF]
                            .rearrange("p (i d) -> p i d", i=8),
                        v_sb[:, off(p, h):off(p, h) + 256][:, None, :]
                            .broadcast_to((128, 8, 256)),
                    ).then_inc(vev_p, 1)
                if g + 4 < NCH and (g + 4) % 2 == 0:
                    emit_qbc_even(g + 4)


            wait_znw_upto(gpsimd, 27)
            emit_reload_batch(gpsimd, (dmaep_p, dmaep_pn), 1, 1)
            wait_znw_upto(gpsimd, 31)
            emit_reload_batch(gpsimd, (dmaep_p, dmaep_pn), 2, 1)

        # ---------------- tensor (PE) ----------------
        @block.tensor
        def _(tensor):
            for gi, wname in enumerate(ALL_HEADS):
                src_sb = enca_sb if wname in ("wq1", "wq2") else encb_sb
                enc_sem = dma_ea if wname in ("wq1", "wq2") else dma_eb
                wsem, wcnt = dma_w[wname], 16
                for mt in range(2):
                    grp = gi * 2 + mt
                    tensor.wait_ge(enc_sem, 16)
                    tensor.wait_ge(wsem, wcnt)
                    tensor.wait_ge(vmz, 1)
                    if grp >= 4:
                        tensor.wait_ge(aevac, grp - 3)
                    buf = ps_projs[grp % 4]
                    mm = None
                    for kt in range(KT):
                        mm = nc.tensor.matmul(
                            buf,
                            lhsT=src_sb[:, kt * 256 + mt * 128:
                                        kt * 256 + mt * 128 + 128],
                            rhs=w_sb[:, (gi * KT + kt) * 256:(gi * KT + kt + 1) * 256],
                            start=(kt == 0), stop=(kt == KT - 1),
                        )
                    mm.then_inc(peproj, 1)

            tensor.wait_ge(dma_rs, 16)
            for g in range(NCH):
                tensor.wait_ge(scexp, g + 1)
                tensor.wait_ge(vev_d, ev_dve_prefix(g))
                tensor.wait_ge(vev_p, ev_pool_prefix(g))
                if g in (0, 1):
                    tensor.wait_ge(vmz, 2)        # full memsets of ps4[0], ps4[1]
                if g in (2, 3):
                    tensor.wait_ge(aevac, NGRP)   # proj aliases ps4[2], ps4[3]
                if g >= 4:
                    c = g - 4
                    if c in EVAC_ACT:
                        tensor.wait_ge(sevac_a, evac_act_prefix(c))
                    else:
                        tensor.wait_ge(sevac_d, evac_dve_prefix(c))
                rbuf = ps4[g % 4]
                eoff = (g % 4) * CHF
                mm2 = None
                for t2 in range(4):
                    cb, rp = t2 // 2, t2 % 2
                    cols = cb * 512
                    nc.tensor.matmul(
                        rbuf[64 * rp:64 * rp + 2, cols:cols + 512],
                        lhsT=redsel_sb[:, :],
                        rhs=e_sb[:, eoff + t2 * 512:eoff + (t2 + 1) * 512],
                        start=True, stop=True,
                        tile_position=(0, 64 * rp),
                    )
                    mm2 = nc.tensor.matmul(
                        rbuf[64 * rp + 32:64 * rp + 34, cols:cols + 512],
                        lhsT=redsel_sb[:, :],
                        rhs=ev_sb[:, eoff + t2 * 512:eoff + (t2 + 1) * 512],
                        start=True, stop=True,
                        tile_position=(0, 64 * rp + 32),
                    )
                mm2.then_inc(pered, 1)

        # ---------------- vector (DVE) ----------------
        @block.vector
        def _(vector):
            nc.vector.memset(ps4[2][:, 256:512], 0.0)
            nc.vector.memset(ps4[2][:, 768:1024], 0.0)
            nc.vector.memset(ps4[3][:, 256:512], 0.0)
            nc.vector.memset(ps4[3][:, 768:1024], 0.0).then_inc(vmz, 1)

            def emit_sprod(g):
                p, h, c2 = chunk_params(g)
                gbuf = (g // 2) % 2
                src_sb = qbc_sb if g % 2 == 1 else qbc2_sb
                qsem = qdma_o[gbuf] if g % 2 == 1 else qdma_e[gbuf]
                vector.wait_ge(aevac, k_grp(g))
                vector.wait_ge(qsem, 16 * (g // 4 + 1))
                if g >= 4:
                    vector.wait_ge(scexp, g - 3)
                nc.vector.tensor_mul(
                    s_sb[:, (g % 4) * CHF:(g % 4 + 1) * CHF]
                        .rearrange("p (i d) -> p i d", i=8),
                    k_sb[:, off(p, h):off(p, h) + 256][:, None, :]
                        .broadcast_to((128, 8, 256)),
                    src_sb[:, gbuf * CHF:(gbuf + 1) * CHF]
                        .rearrange("p (i d) -> p i d", i=8),
                ).then_inc(vstt, 1)

            nc.vector.memset(ps4[0][:, :], 0.0)
            nc.vector.memset(ps4[1][:, :], 0.0).then_inc(vmz, 1)
            for g0 in range(4):
                emit_sprod(g0)
            for g in range(NCH):
                if g + 4 < NCH:
                    emit_sprod(g + 4)
                if g in EV_DVE:
                    p, h, c2 = chunk_params(g)
                    vector.wait_ge(aevac, v_grp(g))
                    vector.wait_ge(scexp, g + 1)
                    if g >= 4:
                        vector.wait_ge(pered, g - 3)
                    nc.vector.tensor_mul(
                        ev_sb[:, (g % 4) * CHF:(g % 4 + 1) * CHF]
                            .rearrange("p (i d) -> p i d", i=8),
                        e_sb[:, (g % 4) * CHF:(g % 4 + 1) * CHF]
                            .rearrange("p (i d) -> p i d", i=8),
                        v_sb[:, off(p, h):off(p, h) + 256][:, None, :]
                            .broadcast_to((128, 8, 256)),
                    ).then_inc(vev_d, 1)
                # DVE evacs for chunk c = g - 2
                if g >= 2 and (g - 2) not in EVAC_ACT:
                    c = g - 2
                    vector.wait_ge(pered, c + 1)
                    if c >= 4:
                        vector.wait_ge(dmazn[c % 4], 16 * (c // 4))
                    nc.vector.tensor_copy(
                        zn_sb[:, (c % 4) * 1024:(c % 4) * 1024 + 1024],
                        ps4[c % 4][0:98, :],
                    ).then_inc(sevac_d, 1)
            for c in (NCH - 2, NCH - 1):
                if c not in EVAC_ACT:
                    vector.wait_ge(pered, c + 1)
                    vector.wait_ge(dmazn[c % 4], 16 * (c // 4))
                    nc.vector.tensor_copy(
                        zn_sb[:, (c % 4) * 1024:(c % 4) * 1024 + 1024],
                        ps4[c % 4][0:98, :],
                    ).then_inc(sevac_d, 1)

            # epilogue (col-split halves, pipelined with out DMA)
            vector.wait_ge(dmaep, 16 * 8)
            vector.wait_ge(dmaep_p, 16 * 4)
            nc.vector.reciprocal(rcp_sb[:, :], z_all[:, :]).then_inc(vep, 1)
            vector.wait_ge(dmaep_n, 16 * 8)
            vector.wait_ge(dmaep_pn, 16 * 4)
            vector.wait_ge(vep, 1)
            nc.vector.tensor_mul(outp_sb[:, 0:512], n_all[:, 0:512], rcp_sb[:, 0:512]
                                 ).then_inc(vep, 2)
            nc.vector.tensor_mul(outp_sb[:, 512:1024], n_all[:, 512:1024],
                                 rcp_sb[:, 512:1024]).then_inc(vep, 3)

    return nc


# ---------------- host side ----------------

def make_inputs(enc_shard, W):
    import ml_dtypes
    bf16 = ml_dtypes.bfloat16

    def ext(w, b):
        m = np.zeros((768, 256), np.float32)
        m[:704] = np.asarray(w, np.float32)
        m[704] = np.asarray(b, np.float32)
        return m.astype(bf16)

    e = np.asarray(enc_shard, np.float32)
    encb = np.zeros((768, 256), np.float32)
    encb[:704] = e.transpose(2, 1, 0).reshape(704, 256)   # col = b*64 + j
    encb[704] = 1.0
    enca = np.zeros((768, 256), np.float32)
    enca[:704] = e.transpose(2, 0, 1).reshape(704, 256)   # col = i*4 + b
    enca[704] = 1.0

    redsel = np.zeros((128, 2), np.float32)
    redsel[:64, 0] = 1.0
    redsel[64:, 1] = 1.0

    return {
        "encb": encb.astype(bf16), "enca": enca.astype(bf16),
        "wk1": ext(W["Wk1"], W["bk1"]), "wv1": ext(W["Wv1"], W["bv1"]),
        "wk2": ext(W["Wk2"], W["bk2"]), "wv2": ext(W["Wv2"], W["bv2"]),
        "wq1": ext(W["Wq1"], W["bq1"]), "wq2": ext(W["Wq2"], W["bq2"]),
        "redsel": redsel.astype(bf16),
    }


_ASSEMBLE_IDX = None


def _assemble_indices():
    """res [128, 2(cb), 2(i2), 256] -> (rowidx [128,2,2], bidx [128])."""
    global _ASSEMBLE_IDX
    if _ASSEMBLE_IDX is not None:
        return _ASSEMBLE_IDX
    P = np.arange(128)
    rp = P // 64
    b2v = (P // 32) % 2
    c = P % 32
    t, c2 = c // 8, c % 8
    p, h = t // 2, t % 2
    cb = np.arange(2)
    i2 = np.arange(2)
    i8 = 2 * (cb[None, :, None] * 2 + rp[:, None, None]) + i2[None, None, :]
    rowidx = h[:, None, None] * 64 + c2[:, None, None] * 8 + i8   # [128, 2, 2]
    bidx = 2 * p + b2v                                             # [128]
    _ASSEMBLE_IDX = (rowidx.astype(int), bidx.astype(int))
    return _ASSEMBLE_IDX


def assemble_output(res_out, core, full_out):
    rowidx, bidx = _assemble_indices()
    r4 = np.asarray(res_out, np.float32).reshape(128, 2, 2, 256)
    full_out[rowidx, (core * 4 + bidx)[:, None, None], :] = r4


# ======================================================================

_NC_CACHE = {}


def _get_nc():
    if "nc" not in _NC_CACHE:
        _NC_CACHE["nc"] = build_nc()
    return _NC_CACHE["nc"]


def kernel(encodings, Wk1, bk1, Wk2, bk2, Wv1, bv1, Wv2, bv2, Wq1, bq1, Wq2, bq2):
    from concourse.bass_utils import run_bass_kernel_spmd

    W = {"Wk1": Wk1, "bk1": bk1, "Wk2": Wk2, "bk2": bk2,
         "Wv1": Wv1, "bv1": bv1, "Wv2": Wv2, "bv2": bv2,
         "Wq1": Wq1, "bq1": bq1, "Wq2": Wq2, "bq2": bq2}
    enc = np.asarray(encodings, np.float32)
    in_maps = []
    for core in range(8):
        shard = enc[:, core * 4:(core + 1) * 4, :]
        in_maps.append(make_inputs(shard, W))

    nc = _get_nc()
    res = run_bass_kernel_spmd(nc, in_maps, core_ids=list(range(8)))

    full = np.zeros((128, 32, 256), np.float32)
    for core in range(8):
        assemble_output(res.results[core]["out"], core, full)
    return full
